# revision 14
# baseline (speedup 1.0000x reference)
"""Multi-head attention (projections + masked softmax + context + fc + residual
LayerNorm) as a Bass/Tile kernel for Trainium2, run SPMD on 8 NeuronCores.

Sharding: core c handles batch b = c//2 and query-half qh = c%2 (1024 of the
2048 query rows, all 8 heads).  Each core computes K/V projections for its
full batch (replicated across the 2 cores of a batch) and everything else for
its own query rows, so the fc output needs no cross-core reduction and no
collectives are required.

Per-core kernel outline (all shapes per core):
  phase 1:  transpose inputs on PE (fp32 has no DMA transpose), project
            Q^T [512,1024], K^T [512,2048] (e on partitions; head pair 2j/2j+1
            on partition halves 0-63 / 64-127 of e-tile j) and V [2048,512].
  phase 2:  per (qtile, head): scores into PSUM as (identity @ mask_neg)
            start=True followed by Q^T-block @ K^T accumulate (mask injected
            by the PE itself, paired heads row-tiled on the 128x128 array);
            exp on ScalarE with accum_out giving softmax row sums for free;
            in-place normalize on VectorE; DMA the attention probabilities
            out; PE-transpose them and matmul with V (paired heads col-tiled)
            for context^T; stack, fc matmul, residual add + LayerNorm
            (bn_stats/bn_aggr, rsqrt via exp(-0.5*ln(var+eps)) to stay in one
            ACT table set), DMA out.
"""

import numpy as np

B, S, DM, H, DK = 4, 2048, 512, 8, 64
SQ = S // 2        # query rows per core
P = 128
NEG = -1.0e9
SCALE = 0.125      # 1/sqrt(DK)
LN_EPS = 1e-5
N_CORES = 8

QT_TILES = SQ // P      # 8 query tiles per core
KCH = S // 512          # 4 k-chunks of 512

_cache = {}


def _build():
    from contextlib import ExitStack

    import concourse.bass as bass
    import concourse.mybir as mybir
    import concourse.tile as tile
    f32 = mybir.dt.float32
    u8 = mybir.dt.uint8
    Alu = mybir.AluOpType
    Act = mybir.ActivationFunctionType

    nc = bass.Bass("TRN2")

    xq = nc.dram_tensor("xq", (SQ, DM), f32, kind="ExternalInput")
    xk = nc.dram_tensor("xk", (S, DM), f32, kind="ExternalInput")
    xv = nc.dram_tensor("xv", (S, DM), f32, kind="ExternalInput")
    msk = nc.dram_tensor("msk", (SQ, S), u8, kind="ExternalInput")
    wq = nc.dram_tensor("wq", (DM, DM), f32, kind="ExternalInput")
    wk = nc.dram_tensor("wk", (DM, DM), f32, kind="ExternalInput")
    wv = nc.dram_tensor("wv", (DM, DM), f32, kind="ExternalInput")
    wfc = nc.dram_tensor("wfc", (DM, DM), f32, kind="ExternalInput")
    gamma = nc.dram_tensor("gamma", (DM,), f32, kind="ExternalInput")
    beta = nc.dram_tensor("beta", (DM,), f32, kind="ExternalInput")
    # identity loaded from DRAM: building it on GPSIMD gives the first PE
    # transposes one cross-engine wait too many for the LDWEIGHTS sync slots
    ident_d = nc.dram_tensor("ident", (P, P), f32, kind="ExternalInput")
    attn_o = nc.dram_tensor("attn_o", (H, SQ, S), f32, kind="ExternalOutput")
    out_o = nc.dram_tensor("out_o", (SQ, DM), f32, kind="ExternalOutput")

    copy_ctr = [0]

    def copy_split(out, in_, act_weight=3, period=8):
        """Spread PSUM->SBUF copies across ScalarE and VectorE."""
        i = copy_ctr[0] % period
        copy_ctr[0] += 1
        if i < act_weight:
            nc.scalar.copy(out, in_)
        else:
            nc.vector.tensor_copy(out, in_)

    with tile.TileContext(nc) as tc, ExitStack() as ctx:
        const = ctx.enter_context(tc.tile_pool(name="const", bufs=1))
        persist = ctx.enter_context(tc.tile_pool(name="persist", bufs=1))

        ident = const.tile([P, P], f32)
        nc.sync.dma_start(ident, ident_d[:, :])
        bf16 = mybir.dt.bfloat16
        fp16 = mybir.dt.float16
        ident_bf = const.tile([P, P], bf16)
        nc.vector.tensor_copy(ident_bf, ident)
        def bcast_row(dram_vec):
            ap = dram_vec[:]
            return bass.AP(
                tensor=ap.tensor, offset=ap.offset, ap=[[0, P]] + list(ap.ap))

        gamma_b = const.tile([P, DM], f32)
        nc.gpsimd.dma_start(out=gamma_b, in_=bcast_row(gamma))
        beta_b = const.tile([P, DM], f32)
        nc.gpsimd.dma_start(out=beta_b, in_=bcast_row(beta))
        eps_t = const.tile([P, 1], f32)
        nc.vector.memset(eps_t, LN_EPS)

        wfc_sb = persist.tile([P, 4, DM], f32)
        nc.sync.dma_start(wfc_sb, wfc.rearrange("(kt p) e -> p kt e", p=P))
        wfc_bf = persist.tile([P, 4, DM], bf16)
        nc.vector.tensor_copy(wfc_bf, wfc_sb)
        xq_nat = persist.tile([P, QT_TILES, DM], f32)
        nc.sync.dma_start(xq_nat, xq.rearrange("(t p) e -> p t e", p=P))

        # Projected tensors, transposed: element [p, j, s] = X^T[j*128+p, s].
        # Head h lives at partitions 64*(h%2)..+64 of e-tile j = h//2.
        # Q^T/K^T kept as fp16 hi+lo pairs: the scores matmul runs three
        # half-precision passes (hi*hi + hi*lo + lo*hi, error ~2^-22) at
        # full PE rate instead of fp32's two serialized-weight-load passes.
        QTh = persist.tile([P, 4, SQ], fp16)
        QTl = persist.tile([P, 4, SQ], fp16)
        KTh = persist.tile([P, 4, S], fp16)
        KTl = persist.tile([P, 4, S], fp16)
        Vt = persist.tile([P, S // P, DM], bf16)  # V natural, bf16 for context

        # Mask pools opened before phase 1 so mask prep overlaps projections.
        mskp = ctx.enter_context(tc.tile_pool(name="mskp", bufs=2))
        mnegp = ctx.enter_context(tc.tile_pool(name="mnegp", bufs=2))

        # ---------------- phase 1: transposes + projections ----------------
        with ExitStack() as p1:
            wpool = p1.enter_context(tc.tile_pool(name="wpool", bufs=1))
            stage = p1.enter_context(tc.tile_pool(name="stage", bufs=6))
            xtp = p1.enter_context(tc.tile_pool(name="xtp", bufs=2))
            tpsum = p1.enter_context(
                tc.tile_pool(name="tpsum", bufs=2, space="PSUM"))
            ppsum = p1.enter_context(
                tc.tile_pool(name="ppsum", bufs=4, space="PSUM"))

            wq_sb = wpool.tile([P, 4, DM], f32)
            nc.sync.dma_start(wq_sb, wq.rearrange("(kt p) e -> p kt e", p=P))
            wk_sb = wpool.tile([P, 4, DM], f32)
            nc.sync.dma_start(wk_sb, wk.rearrange("(kt p) e -> p kt e", p=P))
            wv_sb = wpool.tile([P, 4, DM], f32)
            nc.sync.dma_start(wv_sb, wv.rearrange("(kt p) e -> p kt e", p=P))
            wv_bf = wpool.tile([P, 4, DM], bf16)
            nc.vector.tensor_copy(wv_bf, wv_sb)

            def split_hilo(hi, lo, pp):
                copy_split(hi, pp)
                nc.vector.tensor_tensor(lo, pp, hi, Alu.subtract)

            def transpose_chunk(nat_tiles, dtype=f32):
                """nat_tiles: 4 [128, 512] natural s-tiles -> xT [128,4,512]."""
                idt = ident if dtype == f32 else ident_bf
                xT = xtp.tile([P, 4, 512], dtype, tag=f"xT{dtype}", name="xT")
                for kt in range(4):
                    tp = tpsum.tile([P, 512], dtype, tag=f"tp{dtype}", name="tp")
                    for ss in range(4):
                        nc.tensor.transpose(
                            tp[:, ss * P:(ss + 1) * P],
                            nat_tiles[ss][:, kt * P:(kt + 1) * P],
                            idt,
                        )
                    copy_split(xT[:, kt, :], tp)
                return xT

            # input_K -> K^T  (out[e,s], lhsT = W_K block, rhs = xk^T chunk)
            for ch in range(KCH):
                nats = []
                for ss in range(4):
                    st = ch * 4 + ss
                    xn = stage.tile([P, DM], f32, tag="stage", name="xn")
                    nc.sync.dma_start(xn, xk[st * P:(st + 1) * P, :])
                    nats.append(xn)
                xT = transpose_chunk(nats)
                for j in range(4):
                    pp = ppsum.tile([P, 512], f32, tag="pp", name="pp")
                    for kt in range(4):
                        nc.tensor.matmul(
                            pp, wk_sb[:, kt, j * P:(j + 1) * P], xT[:, kt, :],
                            start=(kt == 0), stop=(kt == 3))
                    split_hilo(KTh[:, j, ch * 512:(ch + 1) * 512],
                               KTl[:, j, ch * 512:(ch + 1) * 512], pp)

            # input_V -> V natural, all in bf16 (context-only precision)
            for ch in range(KCH):
                nats = []
                for ss in range(4):
                    st = ch * 4 + ss
                    xn = stage.tile([P, DM], f32, tag="stage", name="xn")
                    nc.sync.dma_start(xn, xv[st * P:(st + 1) * P, :])
                    xb = stage.tile([P, DM], bf16, tag="stageb", name="xb")
                    nc.vector.tensor_copy(xb, xn)
                    nats.append(xb)
                xT = transpose_chunk(nats, dtype=bf16)
                for ss in range(4):
                    st = ch * 4 + ss
                    pp = ppsum.tile([P, 512], f32, tag="pp", name="pp")
                    for kt in range(4):
                        nc.tensor.matmul(
                            pp, xT[:, kt, ss * P:(ss + 1) * P], wv_bf[:, kt, :],
                            start=(kt == 0), stop=(kt == 3))
                    copy_split(Vt[:, st, :], pp)

            # input_Q -> Q^T
            for ch in range(SQ // 512):
                nats = [xq_nat[:, ch * 4 + ss, :] for ss in range(4)]
                xT = transpose_chunk(nats)
                for j in range(4):
                    pp = ppsum.tile([P, 512], f32, tag="pp", name="pp")
                    for kt in range(4):
                        nc.tensor.matmul(
                            pp, wq_sb[:, kt, j * P:(j + 1) * P], xT[:, kt, :],
                            start=(kt == 0), stop=(kt == 3))
                    split_hilo(QTh[:, j, ch * 512:(ch + 1) * 512],
                               QTl[:, j, ch * 512:(ch + 1) * 512], pp)

        # ---------------- phase 2: attention ----------------
        epool = ctx.enter_context(tc.tile_pool(name="epool", bufs=3))
        abfpool = ctx.enter_context(tc.tile_pool(name="abfpool", bufs=2))
        atpool = ctx.enter_context(tc.tile_pool(name="atpool", bufs=2))
        ctxp = ctx.enter_context(tc.tile_pool(name="ctxp", bufs=2))
        xpool = ctx.enter_context(tc.tile_pool(name="xpool", bufs=2))
        small = ctx.enter_context(tc.tile_pool(name="small", bufs=4))
        spool = ctx.enter_context(tc.tile_pool(name="spool", bufs=2, space="PSUM"))
        etpsum = ctx.enter_context(tc.tile_pool(name="etpsum", bufs=2, space="PSUM"))
        ctpsum = ctx.enter_context(tc.tile_pool(name="ctpsum", bufs=1, space="PSUM"))
        fcpsum = ctx.enter_context(tc.tile_pool(name="fcpsum", bufs=1, space="PSUM"))

        for qt in range(QT_TILES):
            msk_t = mskp.tile([P, S], u8, tag="msk", name="msk_t")
            nc.sync.dma_start(msk_t, msk[qt * P:(qt + 1) * P, :])
            mneg = mnegp.tile([P, S], bf16, tag="mneg", name="mneg")
            nc.vector.tensor_scalar_mul(mneg, msk_t, NEG)

            ctxT = ctxp.tile([P, 4 * P], bf16, tag="ctxT", name="ctxT")

            for j in range(4):  # head pairs (2j, 2j+1)
                heads = (2 * j, 2 * j + 1)
                sums = small.tile([P, 4], f32, tag="sums", name="sums")
                E = []
                for i in range(2):
                    Eh = epool.tile([P, S], f32, tag="E", name="Eh")
                    E.append(Eh)
                # scores + mask inject, halves interleaved across the pair so
                # the K=64 score matmuls row-tile (rows 0-63 / 64-127).
                for hf in range(2):
                    sp = [
                        spool.tile([P, 1024], f32, tag="sp", name="sp")
                        for _ in range(2)
                    ]
                    for c in range(2):
                        off = hf * 1024 + c * 512
                        for i in range(2):
                            nc.tensor.matmul(
                                sp[i][:, c * 512:(c + 1) * 512], ident_bf,
                                mneg[:, off:off + 512],
                                start=True, stop=False)
                        qsl = (slice(64 * 0, 64 * 1), j, slice(qt * P, (qt + 1) * P))
                        for i in range(2):
                            psl = slice(i * 64, (i + 1) * 64)
                            qs = (psl, j, slice(qt * P, (qt + 1) * P))
                            ks = (psl, j, slice(off, off + 512))
                            for mi, (qa, ka) in enumerate(
                                    ((QTh, KTh), (QTh, KTl), (QTl, KTh))):
                                nc.tensor.matmul(
                                    sp[i][:, c * 512:(c + 1) * 512],
                                    qa[qs], ka[ks],
                                    start=False, stop=(mi == 2))
                    for i in range(2):
                        nc.scalar.activation(
                            E[i][:, hf * 1024:(hf + 1) * 1024], sp[i],
                            Act.Exp, bias=0.0, scale=SCALE,
                            accum_out=sums[:, 2 * i + hf:2 * i + hf + 1])

                AT = []
                for i in range(2):
                    ssum = small.tile([P, 1], f32, tag="ssum", name="ssum")
                    nc.vector.tensor_tensor(
                        ssum, sums[:, 2 * i:2 * i + 1],
                        sums[:, 2 * i + 1:2 * i + 2], Alu.add)
                    r = small.tile([P, 1], f32, tag="r", name="r")
                    nc.vector.reciprocal(r, ssum)
                    # normalize in place: E becomes the attention probs A
                    nc.vector.tensor_scalar_mul(E[i], E[i], r)
                    nc.sync.dma_start(
                        attn_o[heads[i], qt * P:(qt + 1) * P, :], E[i])
                    # bf16 copy of A, then PE-transpose in bf16 (no fp32
                    # HI/LO split, half the PSUM copy traffic)
                    Abf = abfpool.tile([P, S], bf16, tag="Abf", name="Abf")
                    nc.vector.tensor_copy(Abf, E[i])
                    ATh = atpool.tile([P, S], bf16, tag="AT", name="ATh")
                    for g in range(4):
                        ep = etpsum.tile([P, 512], f32, tag="ep", name="ep")
                        for ss in range(4):
                            kt = g * 4 + ss
                            # transpose as a REGULAR matmul (A_block^T @ I):
                            # pipelines at N-cycle gaps with background
                            # weight loads and keeps the HAM clock warm,
                            # unlike transpose-mode (~265 ns each, cold)
                            nc.tensor.matmul(
                                ep[:, ss * P:(ss + 1) * P],
                                Abf[:, kt * P:(kt + 1) * P], ident_bf,
                                start=True, stop=True)
                        copy_split(ATh[:, g * 512:(g + 1) * 512], ep)
                    AT.append(ATh)

                # context^T for the pair, col-tiled (out partitions 0-63/64-127)
                ct = ctpsum.tile([P, P], f32, tag="ct", name="ct")
                for kt in range(S // P):
                    for i in range(2):
                        # the pair's two groups share one PSUM bank on
                        # disjoint partition halves; the sim's zero-region
                        # tracking is partition-unaware, hence skip_group_check
                        nc.tensor.matmul(
                            ct[i * 64:(i + 1) * 64, :],
                            Vt[:, kt, heads[i] * DK:(heads[i] + 1) * DK],
                            AT[i][:, kt * P:(kt + 1) * P],
                            start=(kt == 0), stop=(kt == S // P - 1),
                            skip_group_check=True)
                copy_split(ctxT[:, j * P:(j + 1) * P], ct)

            # fc + residual + layernorm for this query tile
            fcp = fcpsum.tile([P, DM], f32, tag="fc", name="fcp")
            for kt in range(4):
                nc.tensor.matmul(
                    fcp, ctxT[:, kt * P:(kt + 1) * P], wfc_bf[:, kt, :],
                    start=(kt == 0), stop=(kt == 3))
            x = xpool.tile([P, DM], f32, tag="x", name="x")
            nc.vector.tensor_tensor(x, fcp, xq_nat[:, qt, :], Alu.add)
            stats = small.tile([P, 6], f32, tag="stats", name="stats")
            nc.vector.bn_stats(stats, x)
            mv = small.tile([P, 2], f32, tag="mv", name="mv")
            nc.vector.bn_aggr(mv, stats)
            # rstd = exp(-0.5*ln(var+eps)); ln+exp share one ACT table set
            lnv = small.tile([P, 1], f32, tag="lnv", name="lnv")
            nc.scalar.activation(lnv, mv[:, 1:2], Act.Ln, bias=eps_t)
            rstd = small.tile([P, 1], f32, tag="rstd", name="rstd")
            nc.scalar.activation(rstd, lnv, Act.Exp, bias=0.0, scale=-0.5)
            xn = xpool.tile([P, DM], f32, tag="xn", name="xn")
            nc.vector.tensor_scalar(
                xn, x, mv[:, 0:1], rstd, op0=Alu.subtract, op1=Alu.mult)
            xg = xpool.tile([P, DM], f32, tag="xg", name="xg")
            nc.vector.tensor_tensor(xg, xn, gamma_b, Alu.mult)
            out_t = xpool.tile([P, DM], f32, tag="out_t", name="out_t")
            nc.vector.tensor_tensor(out_t, xg, beta_b, Alu.add)
            nc.sync.dma_start(out_o[qt * P:(qt + 1) * P, :], out_t)

    return nc


def _legalize_multi_waits(nc, mybir, max_waits=1):
    """Walrus codegen rejects instructions carrying more than one semaphore
    wait ("Too many sync wait commands").  Tile's scheduler happily emits
    several; split the extras into standalone EventSemaphore waits placed
    immediately before the instruction on the same engine queue."""
    n_split = 0
    for f in nc.m.functions:
        for blk in f.blocks:
            changed = False
            newl = []
            for ins in blk.instructions:
                si = ins.sync_info
                if si is not None and len(si.on_wait) > max_waits:
                    waits = list(si.on_wait)
                    for k, w in enumerate(waits[:-max_waits]):
                        es = mybir.InstEventSemaphore(
                            name=f"{ins.name}_hw{k}", ins=[], outs=[],
                            engine=ins.engine)
                        es.sync_info = mybir.SyncInfo(on_wait=[w], on_update=[])
                        newl.append(es)
                        n_split += 1
                    ins.sync_info = mybir.SyncInfo(
                        on_wait=waits[-max_waits:],
                        on_update=list(si.on_update))
                    changed = True
                newl.append(ins)
            if changed:
                blk.instructions = newl
    return n_split


def get_nc():
    """Build (once) and return the walrus-ready module (waits legalized)."""
    if "nc" not in _cache:
        import concourse.mybir as mybir

        nc = _build()
        _legalize_multi_waits(nc, mybir)
        _cache["nc"] = nc
    return _cache["nc"]


def make_in_maps(input_Q, input_K, input_V, attn_mask, W_Q, W_K, W_V, W_fc,
                 ln_gamma, ln_beta):
    input_Q = np.ascontiguousarray(np.asarray(input_Q), dtype=np.float32)
    input_K = np.ascontiguousarray(np.asarray(input_K), dtype=np.float32)
    input_V = np.ascontiguousarray(np.asarray(input_V), dtype=np.float32)
    mask_u8 = np.asarray(attn_mask).astype(np.uint8)
    W_Q = np.ascontiguousarray(np.asarray(W_Q), dtype=np.float32)
    W_K = np.ascontiguousarray(np.asarray(W_K), dtype=np.float32)
    W_V = np.ascontiguousarray(np.asarray(W_V), dtype=np.float32)
    W_fc = np.ascontiguousarray(np.asarray(W_fc), dtype=np.float32)
    ln_gamma = np.ascontiguousarray(np.asarray(ln_gamma), dtype=np.float32)
    ln_beta = np.ascontiguousarray(np.asarray(ln_beta), dtype=np.float32)

    ident = np.eye(P, dtype=np.float32)
    in_maps = []
    for c in range(N_CORES):
        b, qh = c // 2, c % 2
        q0 = qh * SQ
        in_maps.append({
            "xq": np.ascontiguousarray(input_Q[b, q0:q0 + SQ]),
            "xk": input_K[b],
            "xv": input_V[b],
            "msk": np.ascontiguousarray(mask_u8[b, q0:q0 + SQ]),
            "wq": W_Q, "wk": W_K, "wv": W_V, "wfc": W_fc,
            "gamma": ln_gamma, "beta": ln_beta, "ident": ident,
        })
    return in_maps


def assemble(results):
    out = np.empty((B, S, DM), np.float32)
    attn = np.empty((B, H, S, S), np.float32)
    for c in range(N_CORES):
        b, qh = c // 2, c % 2
        q0 = qh * SQ
        out[b, q0:q0 + SQ] = results[c]["out_o"]
        attn[b, :, q0:q0 + SQ, :] = results[c]["attn_o"]
    return out, attn


def run(trace=False, **inputs):
    from concourse.bass_utils import run_bass_kernel_spmd

    nc = get_nc()
    in_maps = make_in_maps(**inputs)
    res = run_bass_kernel_spmd(
        nc, in_maps, core_ids=list(range(N_CORES)), trace=trace)
    return assemble(res.results), res


def kernel(input_Q, input_K, input_V, attn_mask, W_Q, W_K, W_V, W_fc,
           ln_gamma, ln_beta):
    (out, attn), _ = run(
        input_Q=input_Q, input_K=input_K, input_V=input_V,
        attn_mask=attn_mask, W_Q=W_Q, W_K=W_K, W_V=W_V, W_fc=W_fc,
        ln_gamma=ln_gamma, ln_beta=ln_beta)
    return out, attn


# revision 16
# speedup vs baseline: 1.2072x; 1.2072x over previous
"""Multi-head attention (projections + masked softmax + context + fc + residual
LayerNorm) as a Bass/Tile kernel for Trainium2, run SPMD on 8 NeuronCores.

Sharding: core c handles batch b = c//2 and query-half qh = c%2 (1024 of the
2048 query rows, all 8 heads).  Each core computes K/V projections for its
full batch (replicated across the 2 cores of a batch) and everything else for
its own query rows, so the fc output needs no cross-core reduction and no
collectives are required.

Per-core kernel outline (all shapes per core):
  phase 1:  transpose inputs on PE (fp32 has no DMA transpose), project
            Q^T [512,1024], K^T [512,2048] (e on partitions; head pair 2j/2j+1
            on partition halves 0-63 / 64-127 of e-tile j) and V [2048,512].
  phase 2:  per (qtile, head): scores into PSUM as (identity @ mask_neg)
            start=True followed by Q^T-block @ K^T accumulate (mask injected
            by the PE itself, paired heads row-tiled on the 128x128 array);
            exp on ScalarE with accum_out giving softmax row sums for free;
            in-place normalize on VectorE; DMA the attention probabilities
            out; PE-transpose them and matmul with V (paired heads col-tiled)
            for context^T; stack, fc matmul, residual add + LayerNorm
            (bn_stats/bn_aggr, rsqrt via exp(-0.5*ln(var+eps)) to stay in one
            ACT table set), DMA out.
"""

import numpy as np

B, S, DM, H, DK = 4, 2048, 512, 8, 64
SQ = S // 2        # query rows per core
P = 128
NEG = -1.0e9
SCALE = 0.125      # 1/sqrt(DK)
LN_EPS = 1e-5
N_CORES = 8

QT_TILES = SQ // P      # 8 query tiles per core
KCH = S // 512          # 4 k-chunks of 512

_cache = {}


def _build():
    from contextlib import ExitStack

    import concourse.bass as bass
    import concourse.mybir as mybir
    import concourse.tile as tile
    f32 = mybir.dt.float32
    u8 = mybir.dt.uint8
    Alu = mybir.AluOpType
    Act = mybir.ActivationFunctionType

    nc = bass.Bass("TRN2")

    xq = nc.dram_tensor("xq", (SQ, DM), f32, kind="ExternalInput")
    xk = nc.dram_tensor("xk", (S, DM), f32, kind="ExternalInput")
    xv = nc.dram_tensor("xv", (S, DM), f32, kind="ExternalInput")
    msk = nc.dram_tensor("msk", (SQ, S), u8, kind="ExternalInput")
    wq = nc.dram_tensor("wq", (DM, DM), f32, kind="ExternalInput")
    wk = nc.dram_tensor("wk", (DM, DM), f32, kind="ExternalInput")
    bf16_d = mybir.dt.bfloat16
    wv = nc.dram_tensor("wv", (DM, DM), bf16_d, kind="ExternalInput")
    wfc = nc.dram_tensor("wfc", (DM, DM), bf16_d, kind="ExternalInput")
    gamma = nc.dram_tensor("gamma", (DM,), f32, kind="ExternalInput")
    beta = nc.dram_tensor("beta", (DM,), f32, kind="ExternalInput")
    # identity loaded from DRAM: building it on GPSIMD gives the first PE
    # transposes one cross-engine wait too many for the LDWEIGHTS sync slots
    ident_d = nc.dram_tensor("ident", (P, P), f32, kind="ExternalInput")
    ident_bf_d = nc.dram_tensor("ident_bf", (P, P), bf16_d, kind="ExternalInput")
    attn_o = nc.dram_tensor("attn_o", (H, SQ, S), f32, kind="ExternalOutput")
    out_o = nc.dram_tensor("out_o", (SQ, DM), f32, kind="ExternalOutput")

    copy_ctr = [0]

    def copy_split(out, in_, act_weight=3, period=8):
        """Spread PSUM->SBUF copies across ScalarE and VectorE."""
        i = copy_ctr[0] % period
        copy_ctr[0] += 1
        if i < act_weight:
            nc.scalar.copy(out, in_)
        else:
            nc.vector.tensor_copy(out, in_)

    with tile.TileContext(nc) as tc, ExitStack() as ctx:
        const = ctx.enter_context(tc.tile_pool(name="const", bufs=1))
        persist = ctx.enter_context(tc.tile_pool(name="persist", bufs=1))

        ident = const.tile([P, P], f32)
        nc.sync.dma_start(ident, ident_d[:, :])
        bf16 = mybir.dt.bfloat16
        fp16 = mybir.dt.float16
        ident_bf = const.tile([P, P], bf16)
        nc.sync.dma_start(ident_bf, ident_bf_d[:, :])
        def bcast_row(dram_vec):
            ap = dram_vec[:]
            return bass.AP(
                tensor=ap.tensor, offset=ap.offset, ap=[[0, P]] + list(ap.ap))

        gamma_b = const.tile([P, DM], f32)
        nc.gpsimd.dma_start(out=gamma_b, in_=bcast_row(gamma))
        beta_b = const.tile([P, DM], f32)
        nc.gpsimd.dma_start(out=beta_b, in_=bcast_row(beta))
        eps_t = const.tile([P, 1], f32)
        nc.vector.memset(eps_t, LN_EPS)

        wfc_bf = persist.tile([P, 4, DM], bf16)
        nc.sync.dma_start(wfc_bf, wfc.rearrange("(kt p) e -> p kt e", p=P))

        # Projected tensors, transposed: element [p, j, s] = X^T[j*128+p, s].
        # Head h lives at partitions 64*(h%2)..+64 of e-tile j = h//2.
        # Q^T/K^T kept as fp16 hi+lo pairs: the scores matmul runs three
        # half-precision passes (hi*hi + hi*lo + lo*hi, error ~2^-22) at
        # full PE rate instead of fp32's two serialized-weight-load passes.
        QTh = persist.tile([P, 4, SQ], fp16)
        KTl = persist.tile([P, 4, S], fp16)
        # Combined stationaries/moving for the fused hi+lo score pass:
        # Qc[:, h] = [Qh_h ; Ql_h] (128 rows), Kc[:, h] = [Kh_h ; Kh_h], so
        # one K=128 matmul computes (Qh+Ql)*Kh; the Qh*Kl pass stays K=64
        # and row-tiles across the head pair.
        Qc = persist.tile([P, H, SQ], fp16)
        Kc = persist.tile([P, H, S], fp16)
        Vt = persist.tile([P, S // P, DM], bf16)  # V natural, bf16 for context

        # Mask pools opened before phase 1 so mask prep overlaps projections.
        mskp = ctx.enter_context(tc.tile_pool(name="mskp", bufs=2))
        mnegp = ctx.enter_context(tc.tile_pool(name="mnegp", bufs=2))

        # ---------------- phase 1: transposes + projections ----------------
        with ExitStack() as p1:
            wpool = p1.enter_context(tc.tile_pool(name="wpool", bufs=1))
            stage = p1.enter_context(tc.tile_pool(name="stage", bufs=5))
            xtp = p1.enter_context(tc.tile_pool(name="xtp", bufs=2))
            tpsum = p1.enter_context(
                tc.tile_pool(name="tpsum", bufs=2, space="PSUM"))
            ppsum = p1.enter_context(
                tc.tile_pool(name="ppsum", bufs=4, space="PSUM"))

            QTl = wpool.tile([P, 4, SQ], fp16)
            KTh = wpool.tile([P, 4, S], fp16)
            wq_sb = wpool.tile([P, 4, DM], f32)
            nc.sync.dma_start(wq_sb, wq.rearrange("(kt p) e -> p kt e", p=P))
            wk_sb = wpool.tile([P, 4, DM], f32)
            nc.sync.dma_start(wk_sb, wk.rearrange("(kt p) e -> p kt e", p=P))
            wv_bf = wpool.tile([P, 4, DM], bf16)
            nc.sync.dma_start(wv_bf, wv.rearrange("(kt p) e -> p kt e", p=P))

            def split_hilo(hi, lo, pp):
                copy_split(hi, pp)
                nc.vector.tensor_tensor(lo, pp, hi, Alu.subtract)

            def transpose_chunk(nat_tiles, dtype=f32):
                """nat_tiles: 4 [128, 512] natural s-tiles -> xT [128,4,512]."""
                idt = ident if dtype == f32 else ident_bf
                xT = xtp.tile([P, 4, 512], dtype, tag=f"xT{dtype}", name="xT",
                              bufs=(2 if dtype == f32 else 1))
                for kt in range(4):
                    tp = tpsum.tile([P, 512], dtype, tag=f"tp{dtype}", name="tp")
                    for ss in range(4):
                        nc.tensor.transpose(
                            tp[:, ss * P:(ss + 1) * P],
                            nat_tiles[ss][:, kt * P:(kt + 1) * P],
                            idt,
                        )
                    copy_split(xT[:, kt, :], tp)
                return xT

            # input_K -> K^T  (out[e,s], lhsT = W_K block, rhs = xk^T chunk)
            for ch in range(KCH):
                nats = []
                for ss in range(4):
                    st = ch * 4 + ss
                    xn = stage.tile([P, DM], f32, tag="stage", name="xn")
                    nc.sync.dma_start(xn, xk[st * P:(st + 1) * P, :])
                    nats.append(xn)
                xT = transpose_chunk(nats)
                for j in range(4):
                    pp = ppsum.tile([P, 512], f32, tag="pp", name="pp")
                    for kt in range(4):
                        nc.tensor.matmul(
                            pp, wk_sb[:, kt, j * P:(j + 1) * P], xT[:, kt, :],
                            start=(kt == 0), stop=(kt == 3))
                    split_hilo(KTh[:, j, ch * 512:(ch + 1) * 512],
                               KTl[:, j, ch * 512:(ch + 1) * 512], pp)

            # input_V -> V natural, all in bf16 (context-only precision)
            for ch in range(KCH):
                nats = []
                for ss in range(4):
                    st = ch * 4 + ss
                    xn = stage.tile([P, DM], f32, tag="stage", name="xn")
                    nc.sync.dma_start(xn, xv[st * P:(st + 1) * P, :])
                    xb = stage.tile([P, DM], bf16, tag="stageb", name="xb")
                    nc.vector.tensor_copy(xb, xn)
                    nats.append(xb)
                xT = transpose_chunk(nats, dtype=bf16)
                for ss in range(4):
                    st = ch * 4 + ss
                    pp = ppsum.tile([P, 512], f32, tag="pp", name="pp")
                    for kt in range(4):
                        nc.tensor.matmul(
                            pp, xT[:, kt, ss * P:(ss + 1) * P], wv_bf[:, kt, :],
                            start=(kt == 0), stop=(kt == 3))
                    copy_split(Vt[:, st, :], pp)

            # input_Q -> Q^T
            for ch in range(SQ // 512):
                nats = []
                for ss in range(4):
                    st = ch * 4 + ss
                    xn = stage.tile([P, DM], f32, tag="stage", name="xn")
                    nc.sync.dma_start(xn, xq[st * P:(st + 1) * P, :])
                    nats.append(xn)
                xT = transpose_chunk(nats)
                for j in range(4):
                    pp = ppsum.tile([P, 512], f32, tag="pp", name="pp")
                    for kt in range(4):
                        nc.tensor.matmul(
                            pp, wq_sb[:, kt, j * P:(j + 1) * P], xT[:, kt, :],
                            start=(kt == 0), stop=(kt == 3))
                    split_hilo(QTh[:, j, ch * 512:(ch + 1) * 512],
                               QTl[:, j, ch * 512:(ch + 1) * 512], pp)

            # assemble the fused-score operands (partition moves need DMA)
            for h in range(H):
                r0 = 64 * (h % 2)
                jj = h // 2
                nc.sync.dma_start(Qc[0:64, h, :], QTh[r0:r0 + 64, jj, :])
                nc.sync.dma_start(Qc[64:128, h, :], QTl[r0:r0 + 64, jj, :])
                nc.sync.dma_start(Kc[0:64, h, :], KTh[r0:r0 + 64, jj, :])
                nc.sync.dma_start(Kc[64:128, h, :], KTh[r0:r0 + 64, jj, :])

        # ---------------- phase 2: attention ----------------
        epool = ctx.enter_context(tc.tile_pool(name="epool", bufs=3))
        abfpool = ctx.enter_context(tc.tile_pool(name="abfpool", bufs=2))
        atpool = ctx.enter_context(tc.tile_pool(name="atpool", bufs=2))
        ctxp = ctx.enter_context(tc.tile_pool(name="ctxp", bufs=2))
        xpool = ctx.enter_context(tc.tile_pool(name="xpool", bufs=2))
        small = ctx.enter_context(tc.tile_pool(name="small", bufs=4))
        spool = ctx.enter_context(tc.tile_pool(name="spool", bufs=2, space="PSUM"))
        etpsum = ctx.enter_context(tc.tile_pool(name="etpsum", bufs=2, space="PSUM"))
        ctpsum = ctx.enter_context(tc.tile_pool(name="ctpsum", bufs=1, space="PSUM"))
        fcpsum = ctx.enter_context(tc.tile_pool(name="fcpsum", bufs=1, space="PSUM"))

        for qt in range(QT_TILES):
            msk_t = mskp.tile([P, S], u8, tag="msk", name="msk_t")
            nc.sync.dma_start(msk_t, msk[qt * P:(qt + 1) * P, :])
            mneg = mnegp.tile([P, S], bf16, tag="mneg", name="mneg")
            nc.vector.tensor_scalar_mul(mneg, msk_t, NEG)

            ctxT = ctxp.tile([P, 4 * P], bf16, tag="ctxT", name="ctxT")

            for j in range(4):  # head pairs (2j, 2j+1)
                heads = (2 * j, 2 * j + 1)
                sums = small.tile([P, 4], f32, tag="sums", name="sums")
                E = []
                for i in range(2):
                    Eh = epool.tile([P, S], f32, tag="E", name="Eh")
                    E.append(Eh)
                # scores + mask inject, halves interleaved across the pair so
                # the K=64 score matmuls row-tile (rows 0-63 / 64-127).
                for hf in range(2):
                    sp = [
                        spool.tile([P, 1024], f32, tag="sp", name="sp")
                        for _ in range(2)
                    ]
                    for c in range(2):
                        off = hf * 1024 + c * 512
                        csl = slice(c * 512, (c + 1) * 512)
                        for i in range(2):
                            nc.tensor.matmul(
                                sp[i][:, csl], ident_bf,
                                mneg[:, off:off + 512],
                                start=True, stop=False)
                        for i in range(2):
                            # fused (Qh+Ql)*Kh, K=128
                            nc.tensor.matmul(
                                sp[i][:, csl],
                                Qc[:, heads[i], qt * P:(qt + 1) * P],
                                Kc[:, heads[i], off:off + 512],
                                start=False, stop=False)
                        for i in range(2):
                            # Qh*Kl, K=64, row-tiled across the pair
                            psl = slice(i * 64, (i + 1) * 64)
                            nc.tensor.matmul(
                                sp[i][:, csl],
                                QTh[psl, j, qt * P:(qt + 1) * P],
                                KTl[psl, j, off:off + 512],
                                start=False, stop=True)
                    for i in range(2):
                        nc.scalar.activation(
                            E[i][:, hf * 1024:(hf + 1) * 1024], sp[i],
                            Act.Exp, bias=0.0, scale=SCALE,
                            accum_out=sums[:, 2 * i + hf:2 * i + hf + 1])

                AT = []
                for i in range(2):
                    ssum = small.tile([P, 1], f32, tag="ssum", name="ssum")
                    nc.vector.tensor_tensor(
                        ssum, sums[:, 2 * i:2 * i + 1],
                        sums[:, 2 * i + 1:2 * i + 2], Alu.add)
                    r = small.tile([P, 1], f32, tag="r", name="r")
                    nc.vector.reciprocal(r, ssum)
                    # normalized probs twice from E: fp32 (DRAM output) and
                    # bf16 (transpose input) - both tensor_scalars run in the
                    # DVE 2x mode, cheaper than one TS plus a 1x CAST
                    Abf = abfpool.tile([P, S], bf16, tag="Abf", name="Abf")
                    nc.vector.tensor_scalar_mul(Abf, E[i], r)
                    nc.vector.tensor_scalar_mul(E[i], E[i], r)
                    nc.sync.dma_start(
                        attn_o[heads[i], qt * P:(qt + 1) * P, :], E[i])
                    ATh = atpool.tile([P, S], bf16, tag="AT", name="ATh")
                    for g in range(4):
                        ep = etpsum.tile([P, 512], f32, tag="ep", name="ep")
                        for ss in range(4):
                            kt = g * 4 + ss
                            # transpose as a REGULAR matmul (A_block^T @ I):
                            # pipelines at N-cycle gaps with background
                            # weight loads and keeps the HAM clock warm,
                            # unlike transpose-mode (~265 ns each, cold)
                            nc.tensor.matmul(
                                ep[:, ss * P:(ss + 1) * P],
                                Abf[:, kt * P:(kt + 1) * P], ident_bf,
                                start=True, stop=True)
                        copy_split(ATh[:, g * 512:(g + 1) * 512], ep)
                    AT.append(ATh)

                # context^T for the pair, col-tiled (out partitions 0-63/64-127)
                ct = ctpsum.tile([P, P], f32, tag="ct", name="ct")
                for kt in range(S // P):
                    for i in range(2):
                        # the pair's two groups share one PSUM bank on
                        # disjoint partition halves; the sim's zero-region
                        # tracking is partition-unaware, hence skip_group_check
                        nc.tensor.matmul(
                            ct[i * 64:(i + 1) * 64, :],
                            Vt[:, kt, heads[i] * DK:(heads[i] + 1) * DK],
                            AT[i][:, kt * P:(kt + 1) * P],
                            start=(kt == 0), stop=(kt == S // P - 1),
                            skip_group_check=True)
                copy_split(ctxT[:, j * P:(j + 1) * P], ct)

            # fc + residual + layernorm for this query tile
            fcp = fcpsum.tile([P, DM], f32, tag="fc", name="fcp")
            for kt in range(4):
                nc.tensor.matmul(
                    fcp, ctxT[:, kt * P:(kt + 1) * P], wfc_bf[:, kt, :],
                    start=(kt == 0), stop=(kt == 3))
            xqr = xpool.tile([P, DM], f32, tag="xqr", name="xqr")
            nc.sync.dma_start(xqr, xq[qt * P:(qt + 1) * P, :])
            x = xpool.tile([P, DM], f32, tag="x", name="x")
            nc.vector.tensor_tensor(x, fcp, xqr, Alu.add)
            stats = small.tile([P, 6], f32, tag="stats", name="stats")
            nc.vector.bn_stats(stats, x)
            mv = small.tile([P, 2], f32, tag="mv", name="mv")
            nc.vector.bn_aggr(mv, stats)
            # rstd = exp(-0.5*ln(var+eps)); ln+exp share one ACT table set
            lnv = small.tile([P, 1], f32, tag="lnv", name="lnv")
            nc.scalar.activation(lnv, mv[:, 1:2], Act.Ln, bias=eps_t)
            rstd = small.tile([P, 1], f32, tag="rstd", name="rstd")
            nc.scalar.activation(rstd, lnv, Act.Exp, bias=0.0, scale=-0.5)
            xn = xpool.tile([P, DM], f32, tag="xn", name="xn")
            nc.vector.tensor_scalar(
                xn, x, mv[:, 0:1], rstd, op0=Alu.subtract, op1=Alu.mult)
            xg = xpool.tile([P, DM], f32, tag="xg", name="xg")
            nc.vector.tensor_tensor(xg, xn, gamma_b, Alu.mult)
            out_t = xpool.tile([P, DM], f32, tag="out_t", name="out_t")
            nc.vector.tensor_tensor(out_t, xg, beta_b, Alu.add)
            nc.sync.dma_start(out_o[qt * P:(qt + 1) * P, :], out_t)

    return nc


def _legalize_multi_waits(nc, mybir, max_waits=1):
    """Walrus codegen rejects instructions carrying more than one semaphore
    wait ("Too many sync wait commands").  Tile's scheduler happily emits
    several; split the extras into standalone EventSemaphore waits placed
    immediately before the instruction on the same engine queue."""
    n_split = 0
    for f in nc.m.functions:
        for blk in f.blocks:
            changed = False
            newl = []
            for ins in blk.instructions:
                si = ins.sync_info
                if si is not None and len(si.on_wait) > max_waits:
                    waits = list(si.on_wait)
                    for k, w in enumerate(waits[:-max_waits]):
                        es = mybir.InstEventSemaphore(
                            name=f"{ins.name}_hw{k}", ins=[], outs=[],
                            engine=ins.engine)
                        es.sync_info = mybir.SyncInfo(on_wait=[w], on_update=[])
                        newl.append(es)
                        n_split += 1
                    ins.sync_info = mybir.SyncInfo(
                        on_wait=waits[-max_waits:],
                        on_update=list(si.on_update))
                    changed = True
                newl.append(ins)
            if changed:
                blk.instructions = newl
    return n_split


def get_nc():
    """Build (once) and return the walrus-ready module (waits legalized)."""
    if "nc" not in _cache:
        import concourse.mybir as mybir

        nc = _build()
        _legalize_multi_waits(nc, mybir)
        _cache["nc"] = nc
    return _cache["nc"]


def make_in_maps(input_Q, input_K, input_V, attn_mask, W_Q, W_K, W_V, W_fc,
                 ln_gamma, ln_beta):
    input_Q = np.ascontiguousarray(np.asarray(input_Q), dtype=np.float32)
    input_K = np.ascontiguousarray(np.asarray(input_K), dtype=np.float32)
    input_V = np.ascontiguousarray(np.asarray(input_V), dtype=np.float32)
    mask_u8 = np.asarray(attn_mask).astype(np.uint8)
    W_Q = np.ascontiguousarray(np.asarray(W_Q), dtype=np.float32)
    W_K = np.ascontiguousarray(np.asarray(W_K), dtype=np.float32)
    import ml_dtypes
    W_V = np.ascontiguousarray(np.asarray(W_V, dtype=np.float32).astype(ml_dtypes.bfloat16))
    W_fc = np.ascontiguousarray(np.asarray(W_fc, dtype=np.float32).astype(ml_dtypes.bfloat16))
    ln_gamma = np.ascontiguousarray(np.asarray(ln_gamma), dtype=np.float32)
    ln_beta = np.ascontiguousarray(np.asarray(ln_beta), dtype=np.float32)

    import ml_dtypes
    ident = np.eye(P, dtype=np.float32)
    ident_bf = np.eye(P, dtype=np.float32).astype(ml_dtypes.bfloat16)
    in_maps = []
    for c in range(N_CORES):
        b, qh = c // 2, c % 2
        q0 = qh * SQ
        in_maps.append({
            "xq": np.ascontiguousarray(input_Q[b, q0:q0 + SQ]),
            "xk": input_K[b],
            "xv": input_V[b],
            "msk": np.ascontiguousarray(mask_u8[b, q0:q0 + SQ]),
            "wq": W_Q, "wk": W_K, "wv": W_V, "wfc": W_fc,
            "gamma": ln_gamma, "beta": ln_beta, "ident": ident,
            "ident_bf": ident_bf,
        })
    return in_maps


def assemble(results):
    out = np.empty((B, S, DM), np.float32)
    attn = np.empty((B, H, S, S), np.float32)
    for c in range(N_CORES):
        b, qh = c // 2, c % 2
        q0 = qh * SQ
        out[b, q0:q0 + SQ] = results[c]["out_o"]
        attn[b, :, q0:q0 + SQ, :] = results[c]["attn_o"]
    return out, attn


def run(trace=False, **inputs):
    from concourse.bass_utils import run_bass_kernel_spmd

    nc = get_nc()
    in_maps = make_in_maps(**inputs)
    res = run_bass_kernel_spmd(
        nc, in_maps, core_ids=list(range(N_CORES)), trace=trace)
    return assemble(res.results), res


def kernel(input_Q, input_K, input_V, attn_mask, W_Q, W_K, W_V, W_fc,
           ln_gamma, ln_beta):
    (out, attn), _ = run(
        input_Q=input_Q, input_K=input_K, input_V=input_V,
        attn_mask=attn_mask, W_Q=W_Q, W_K=W_K, W_V=W_V, W_fc=W_fc,
        ln_gamma=ln_gamma, ln_beta=ln_beta)
    return out, attn
